# revision 33
# baseline (speedup 1.0000x reference)
"""Distributed Bass kernel for causal MHA block (B=4,T=2048,C=1024,H=16,D=64).

Sharding: tensor-parallel over head pairs across 8 cores. Core c owns heads
{2c, 2c+1} and computes QKV+attention for all batches for those heads. The
normalized attention outputs (attnT: head-dims on partitions, tokens free)
are AllGather'd per batch; each core then computes the o-projection for its
128 output channels over all tokens (w_o row-sharded), fused with the
residual add. Host reassembles out = concat(outT_c).T.

Layout notes (everything chosen so the device never transposes):
  - scores are computed transposed (keys on partitions, queries free) so the
    probs tile feeds the PV matmul directly as the moving operand.
  - V carries a fused ones-column; the PV matmul then emits the softmax
    denominator as row 64 of the accumulator for free.
  - softmax skips max-subtraction: scores*0.125 for randn inputs are bounded
    (|s|<~10), exp stays well inside fp16/fp32 range.
"""

import os
import sys

import numpy as np

sys.path.insert(0, "/opt/trn_rl_repo")

B, T, C, H, D = 4, 2048, 1024, 16, 64
BT = B * T  # 8192
N_CORES = 8
F16 = None  # set after mybir import
F32 = None

_cache = {}


def _build_graph():
    import concourse.bacc as bacc
    import concourse.bass as bass
    import concourse.mybir as mybir
    import concourse.tile as tile

    f16 = mybir.dt.float16
    bf16 = mybir.dt.bfloat16
    f32 = mybir.dt.float32
    Alu = mybir.AluOpType
    Act = mybir.ActivationFunctionType

    nc = bacc.Bacc("TRN2", target_bir_lowering=False, debug=False,
                   num_devices=N_CORES)

    xT = nc.dram_tensor("xT", [C, BT], bf16, kind="ExternalInput")
    wqkvT = nc.dram_tensor("wqkvT", [C, 384], bf16, kind="ExternalInput")
    woT = nc.dram_tensor("woT", [C, 128], bf16, kind="ExternalInput")
    residT = nc.dram_tensor("residT", [128, BT], f16, kind="ExternalInput")
    masks = nc.dram_tensor("masks", [512, 512], bf16, kind="ExternalInput")
    ident = nc.dram_tensor("ident", [128, 128], bf16, kind="ExternalInput")
    outT = nc.dram_tensor("outT", [128, BT], f16, kind="ExternalOutput")

    RG = [list(range(N_CORES))]

    with tile.TileContext(nc) as tc:
        with (
            tc.tile_pool(name="const", bufs=1) as constp,
            tc.tile_pool(name="dram", bufs=1, space="DRAM") as dramp,
            tc.tile_pool(name="qkvout", bufs=1) as qkvp,
            tc.tile_pool(name="ps_st", bufs=2, space="PSUM") as ps_st,
            tc.tile_pool(name="ps_x", bufs=2, space="PSUM") as ps_x,
            tc.tile_pool(name="ps_at", bufs=2, space="PSUM") as ps_at,
        ):
            # ---- constants ----
            masks_sb = constp.tile([128, 4 * 512], bf16)
            for j in range(4):
                nc.sync.dma_start(out=masks_sb[:, j * 512:(j + 1) * 512],
                                  in_=masks[j * 128:(j + 1) * 128, :])
            wqkvT_sb = constp.tile([128, 8 * 384], bf16)
            for ci in range(8):
                nc.sync.dma_start(out=wqkvT_sb[:, ci * 384:(ci + 1) * 384],
                                  in_=wqkvT[ci * 128:(ci + 1) * 128, :])
            woT_sb = constp.tile([128, 8 * 128], bf16)
            for ci in range(8):
                nc.sync.dma_start(out=woT_sb[:, ci * 128:(ci + 1) * 128],
                                  in_=woT[ci * 128:(ci + 1) * 128, :])
            ident_sb = constp.tile([128, 128], bf16)
            nc.sync.dma_start(out=ident_sb[:], in_=ident[:])


            # ---- persistent QKV outputs ----
            QT_sb = qkvp.tile([128, BT], bf16)    # rows 0:64 head even, 64:128 odd
            KT_sb = qkvp.tile([128, BT], bf16)
            V_sb = qkvp.tile([128, 64 * 130], bf16)  # per 128-tok tile: Ve|1|Vo|1
            V_g = V_sb.rearrange("p (t g) -> p t g", g=130)
            nc.vector.memset(V_g[:, :, 64:65], 1.0)
            nc.vector.memset(V_g[:, :, 129:130], 1.0)

            # ---- ag buffers: halves for b<3, per-qt for the last batch
            # (so only the final small AllGather is exposed in the tail) ----
            ag_in = [[dramp.tile([128, 1024], bf16, name=f"ag_in{b}_{h}")
                      for h in range(2)] for b in range(B - 1)]
            ag_out = [[dramp.tile([1024, 1024], bf16, name=f"ag_out{b}_{h}",
                                  addr_space="Shared") for h in range(2)]
                      for b in range(B - 1)]
            ag_in3 = [dramp.tile([128, 512], bf16, name=f"ag_in3_{qt}")
                      for qt in range(4)]
            ag_out3 = [dramp.tile([1024, 512], bf16, name=f"ag_out3_{qt}",
                                  addr_space="Shared") for qt in range(4)]

            with tc.tile_pool(name="xT", bufs=2) as xtp:
                with (
                    tc.tile_pool(name="pt", bufs=6) as ptp,
                    tc.tile_pool(name="rc", bufs=3) as rcp,
                    tc.tile_pool(name="atn", bufs=4) as atnp,
                    tc.tile_pool(name="rbs", bufs=2) as rbsp,
                    tc.tile_pool(name="ats", bufs=3) as atsp,
                    tc.tile_pool(name="af", bufs=2) as afp,
                    tc.tile_pool(name="vt", bufs=2) as vtp,
                    tc.tile_pool(name="res", bufs=3) as resp,
                    tc.tile_pool(name="os", bufs=4) as osp,
                ):
                    resid_sb = {}
                    xt_sb = {}

                    # ---- emission units -------------------------------
                    # The PE executes its instruction stream in order, so
                    # pure-PE work (QKV projection of the next batch, o-proj
                    # of the previous batch) is chopped into small units and
                    # interleaved into the exp-paced attention stream, where
                    # the PE would otherwise idle waiting on ScalarE.

                    def make_qkv_units(b):
                        tb = b * T
                        units = []

                        def dmas(b=b, tb=tb):
                            xt = xtp.tile([128, 8 * T], bf16, name="xt")
                            xt_sb[b] = xt
                            for w in range(4):
                                for ci in range(8):
                                    nc.sync.dma_start(
                                        out=xt[:, ci * T + w * 512:
                                               ci * T + (w + 1) * 512],
                                        in_=xT[ci * 128:(ci + 1) * 128,
                                               tb + w * 512:tb + (w + 1) * 512])
                            res = resp.tile([128, T], f16, name="res")
                            resid_sb[b] = res
                            nc.sync.dma_start(out=res[:],
                                              in_=residT[:, tb:tb + T])
                        units.append(dmas)

                        state = {}
                        for which, dstname in ((0, "q"), (1, "k")):
                            for tt in range(4):
                                def sub1(b=b, tb=tb, which=which, tt=tt):
                                    ps = ps_x.tile([128, 512], f32, name="x")
                                    state[(which, tt)] = ps
                                    for ci in range(4):
                                        nc.tensor.matmul(
                                            ps[:],
                                            wqkvT_sb[:, ci * 384 + which * 128:
                                                     ci * 384 + which * 128 + 128],
                                            xt_sb[b][:, ci * T + tt * 512:
                                                     ci * T + (tt + 1) * 512],
                                            start=(ci == 0), stop=False)

                                def sub2(b=b, tb=tb, which=which, tt=tt):
                                    ps = state.pop((which, tt))
                                    for ci in range(4, 8):
                                        nc.tensor.matmul(
                                            ps[:],
                                            wqkvT_sb[:, ci * 384 + which * 128:
                                                     ci * 384 + which * 128 + 128],
                                            xt_sb[b][:, ci * T + tt * 512:
                                                     ci * T + (tt + 1) * 512],
                                            start=False, stop=(ci == 7))
                                    dst = QT_sb if which == 0 else KT_sb
                                    nc.vector.tensor_copy(
                                        dst[:, tb + tt * 512:tb + (tt + 1) * 512],
                                        ps[:])
                                units.append(sub1)
                                units.append(sub2)
                        for vt in range(16):
                            def vu(b=b, tb=tb, vt=vt):
                                t64 = b * 16 + vt
                                ps = ps_x.tile([128, 128], f32, name="x")
                                for ci in range(8):
                                    nc.tensor.matmul(
                                        ps[:],
                                        xt_sb[b][:, ci * T + vt * 128:
                                                 ci * T + (vt + 1) * 128],
                                        wqkvT_sb[:, ci * 384 + 256:
                                                 ci * 384 + 384],
                                        start=(ci == 0), stop=(ci == 7))
                                pv = ps.rearrange("p (h e) -> p h e", e=64)
                                dv = V_g[:, t64, :].rearrange(
                                    "p (h e) -> p h e", e=65)[:, :, 0:64]
                                nc.vector.tensor_copy(dv, pv)
                            units.append(vu)
                        return units

                    def make_oproj_parts(b):
                        tb = b * T
                        parts = []
                        opstate = {}
                        for qt in range(4):
                            def af_fn(b=b, qt=qt):
                                af = afp.tile([128, 8 * 512], bf16, name="af")
                                if b == B - 1:
                                    src = ag_out3[qt].rearrange(
                                        "(c p) q -> p c q", p=128)
                                    nc.sync.dma_start(
                                        out=af.rearrange("p (c q) -> p c q",
                                                         q=512),
                                        in_=src[:, :, :])
                                else:
                                    src = ag_out[b][qt // 2].rearrange(
                                        "(c p) q -> p c q", p=128)
                                    nc.sync.dma_start(
                                        out=af.rearrange("p (c q) -> p c q",
                                                         q=512),
                                        in_=src[:, :, (qt % 2) * 512:
                                                (qt % 2) * 512 + 512])
                                opstate[qt] = af

                            def mm_fn(b=b, tb=tb, qt=qt):
                                af = opstate.pop(qt)
                                ps = ps_x.tile([128, 512], f32, name="x")
                                for ci in range(8):
                                    nc.tensor.matmul(
                                        ps[:],
                                        woT_sb[:, ci * 128:(ci + 1) * 128],
                                        af[:, ci * 512:(ci + 1) * 512],
                                        start=(ci == 0), stop=(ci == 7))
                                osb = osp.tile([128, 512], f16, name="os")
                                nc.vector.tensor_add(
                                    osb[:], ps[:],
                                    resid_sb[b][:, qt * 512:(qt + 1) * 512])
                                nc.sync.dma_start(
                                    out=outT[:, tb + qt * 512:
                                             tb + (qt + 1) * 512],
                                    in_=osb[:])
                            parts.append((af_fn, mm_fn))
                        return parts

                    def make_oproj_units(b):
                        units = []
                        for af_fn, mm_fn in make_oproj_parts(b):
                            def u(af_fn=af_fn, mm_fn=mm_fn):
                                af_fn()
                                mm_fn()
                            units.append(u)
                        return units

                    def emit_attention(b, units, force=None):
                        tb = b * T
                        n_slots = 40
                        total = len(units)
                        popped = 0
                        done_kbs = 0

                        def feed(floor=None):
                            nonlocal popped
                            target = (done_kbs * total + n_slots - 1) // n_slots
                            if floor is not None:
                                target = max(target, floor)
                            while popped < min(target, total):
                                fn, min_kb = units[popped]
                                if min_kb > done_kbs:
                                    break
                                fn()
                                popped += 1

                        for qt in range(4):
                            if force and qt in force:
                                feed(floor=force[qt])
                            q0 = tb + qt * 512
                            nkb = 4 * qt + 4
                            ats = atsp.tile([128, 512], bf16, name="ats")

                            def emit_st(kb, qt=qt, q0=q0, tb=tb):
                                k0 = tb + kb * 128
                                st = ps_st.tile([128, 1024], f32, name="st")
                                for half in (0, 1):
                                    p0 = half * 64
                                    nc.tensor.matmul(
                                        st[:, half * 512:half * 512 + 512],
                                        KT_sb[p0:p0 + 64, k0:k0 + 128],
                                        QT_sb[p0:p0 + 64, q0:q0 + 512],
                                        start=True, stop=True)
                                return st

                            at_eo = [ps_at.tile([65, 512], f32, name="at")
                                     for _ in range(2)]
                            sts = [emit_st(0)]
                            if nkb > 1:
                                sts.append(emit_st(1))
                            for kb in range(nkb):
                                t64 = b * 16 + kb
                                diag = kb - 4 * qt
                                st = sts[kb]
                                pt = ptp.tile([128, 1024], bf16, name="pt")
                                nc.scalar.activation(pt[:], st[:], Act.Exp,
                                                     scale=0.125)
                                if diag >= 0:
                                    for half in (0, 1):
                                        nc.vector.tensor_mul(
                                            pt[:, half * 512:half * 512 + 512],
                                            pt[:, half * 512:half * 512 + 512],
                                            masks_sb[:, diag * 512:
                                                     (diag + 1) * 512])
                                if kb + 2 < nkb:
                                    sts.append(emit_st(kb + 2))
                                for half in (0, 1):
                                    nc.tensor.matmul(
                                        at_eo[half][0:65, :],
                                        V_sb[:, t64 * 130 + half * 65:
                                             t64 * 130 + half * 65 + 65],
                                        pt[:, half * 512:half * 512 + 512],
                                        start=(kb == 0), stop=(kb == nkb - 1))
                                done_kbs += 1
                                feed()
                            # normalize + stage for allgather (DVE/GpSimd
                            # only; psum evacuated immediately)
                            for half in (0, 1):
                                p0 = half * 64
                                at = at_eo[half]
                                atn = atnp.tile([65, 512], f32, name="atn")
                                nc.vector.tensor_copy(atn[:], at[0:65, :])
                                den = rcp.tile([1, 512], f32, name="den")
                                nc.vector.tensor_copy(den[:], at[64:65, :])
                                rc = rcp.tile([1, 512], f32, name="rc")
                                nc.vector.reciprocal_approx_fast(rc[:], den[:])
                                rbs = rbsp.tile([64, 512], f32, name="rbs")
                                nc.gpsimd.partition_broadcast(rbs[:], rc[:])
                                nc.vector.tensor_mul(
                                    ats[p0:p0 + 64, :], atn[0:64, :], rbs[:])
                            if b == B - 1:
                                nc.gpsimd.dma_start(out=ag_in3[qt][:],
                                                    in_=ats[:])
                                nc.gpsimd.collective_compute(
                                    "AllGather", Alu.bypass, replica_groups=RG,
                                    ins=[ag_in3[qt].opt()],
                                    outs=[ag_out3[qt].opt()])
                            else:
                                nc.gpsimd.dma_start(
                                    out=ag_in[b][qt // 2][:, (qt % 2) * 512:
                                                          (qt % 2) * 512 + 512],
                                    in_=ats[:])
                                if qt % 2 == 1:
                                    nc.gpsimd.collective_compute(
                                        "AllGather", Alu.bypass,
                                        replica_groups=RG,
                                        ins=[ag_in[b][qt // 2].opt()],
                                        outs=[ag_out[b][qt // 2].opt()])
                        # drain any leftovers
                        while popped < total:
                            units[popped][0]()
                            popped += 1

                    # ---- main schedule --------------------------------
                    # batch 0: emit only the slice of QKV that attention
                    # qt0 needs, feed the rest as units with forced pops at
                    # q-tile boundaries (dependency order).
                    q0units = make_qkv_units(0)
                    for idx in (0, 1, 2, 9, 10, 17, 18, 19, 20):
                        q0units[idx]()
                    rest0 = []
                    for j in (1, 2, 3):
                        rest0 += [q0units[2 * j + 1], q0units[2 * j + 2],
                                  q0units[9 + 2 * j], q0units[10 + 2 * j]]
                        rest0 += q0units[17 + 4 * j:21 + 4 * j]
                    force0 = {1: 8, 2: 16, 3: 24}

                    tailp = make_oproj_parts(B - 1)
                    for b in range(B):
                        units = list(rest0) if b == 0 else []
                        force = force0 if b == 0 else None
                        qkv = make_qkv_units(b + 1) if b + 1 < B else []
                        op = make_oproj_units(b - 1) if b > 0 else []
                        qi = 0
                        for j in range(4):
                            take = min(len(qkv) - qi, 8 + (1 if j == 0 else 0))
                            units += qkv[qi:qi + take]
                            qi += take
                            if j < len(op):
                                units.append(op[j])
                        units += qkv[qi:]
                        units = [(u, 0) for u in units]
                        if b == B - 1:
                            def op3u(qt):
                                def u():
                                    tailp[qt][0]()
                                    tailp[qt][1]()
                                return u
                            units.append((op3u(0), 26))
                            units.append((op3u(1), 30))
                            units.append((tailp[2][0], 37))
                        emit_attention(b, units, force=force)
                    tailp[2][1]()
                    tailp[3][0]()
                    tailp[3][1]()
    nc.compile()
    return nc


def _host_shards(residual, x, w_qkv, w_o):
    import ml_dtypes
    bf16 = ml_dtypes.bfloat16
    xf = np.ascontiguousarray(x.reshape(BT, C).T).astype(bf16)  # (C, BT)
    rf = residual.reshape(BT, C).T                          # (C, BT) view
    woT_full = w_o.T                                        # (C, C) view

    # causal mask tiles: tile j allows key s (0..127) for query q (0..511)
    # when 128*j + s <= q
    jj = np.arange(4)[:, None, None]
    ss = np.arange(128)[None, :, None]
    qq = np.arange(512)[None, None, :]
    masks = ((128 * jj + ss) <= qq).astype(bf16).reshape(512, 512)
    masks = np.ascontiguousarray(masks)
    ident = np.eye(128).astype(bf16)

    in_maps = []
    for c in range(N_CORES):
        r0, r1 = c * 128, (c + 1) * 128
        wq = w_qkv[r0:r1, :]
        wk = w_qkv[C + r0:C + r1, :]
        wv = w_qkv[2 * C + r0:2 * C + r1, :]
        wqkvT = np.ascontiguousarray(
            np.concatenate([wq.T, wk.T, wv.T], axis=1)).astype(bf16)
        in_maps.append({
            "xT": xf,
            "wqkvT": wqkvT,
            "woT": np.ascontiguousarray(woT_full[:, r0:r1]).astype(bf16),
            "residT": np.ascontiguousarray(rf[r0:r1, :]),
            "masks": masks,
            "ident": ident,
        })
    return in_maps


def kernel(residual, x, w_qkv, w_o):
    from concourse.bass_utils import run_bass_kernel_spmd

    residual = np.asarray(residual, dtype=np.float16)
    x = np.asarray(x, dtype=np.float16)
    w_qkv = np.asarray(w_qkv, dtype=np.float16)
    w_o = np.asarray(w_o, dtype=np.float16)

    if "nc" not in _cache:
        _cache["nc"] = _build_graph()
    nc = _cache["nc"]

    in_maps = _host_shards(residual, x, w_qkv, w_o)
    res = run_bass_kernel_spmd(nc, in_maps, core_ids=list(range(N_CORES)),
                               trace=bool(os.environ.get("BASS_TRACE")))
    _cache["last_result"] = res
    outT = np.concatenate([res.results[c]["outT"] for c in range(N_CORES)],
                          axis=0)                           # (C, BT)
    return np.ascontiguousarray(outT.T).reshape(B, T, C)


# revision 34
# speedup vs baseline: 1.0744x; 1.0744x over previous
"""Distributed Bass kernel for causal MHA block (B=4,T=2048,C=1024,H=16,D=64).

Sharding: tensor-parallel over head pairs across 8 cores. Core c owns heads
{2c, 2c+1} and computes QKV+attention for all batches for those heads. The
normalized attention outputs (attnT: head-dims on partitions, tokens free)
are AllGather'd per batch; each core then computes the o-projection for its
128 output channels over all tokens (w_o row-sharded), fused with the
residual add. Host reassembles out = concat(outT_c).T.

Layout notes (everything chosen so the device never transposes):
  - scores are computed transposed (keys on partitions, queries free) so the
    probs tile feeds the PV matmul directly as the moving operand.
  - V carries a fused ones-column; the PV matmul then emits the softmax
    denominator as row 64 of the accumulator for free.
  - softmax skips max-subtraction: scores*0.125 for randn inputs are bounded
    (|s|<~10), exp stays well inside fp16/fp32 range.
"""

import os
import sys

import numpy as np

sys.path.insert(0, "/opt/trn_rl_repo")

B, T, C, H, D = 4, 2048, 1024, 16, 64
BT = B * T  # 8192
N_CORES = 8
F16 = None  # set after mybir import
F32 = None

_cache = {}


def _build_graph():
    import concourse.bacc as bacc
    import concourse.bass as bass
    import concourse.mybir as mybir
    import concourse.tile as tile

    f16 = mybir.dt.float16
    bf16 = mybir.dt.bfloat16
    f32 = mybir.dt.float32
    Alu = mybir.AluOpType
    Act = mybir.ActivationFunctionType

    nc = bacc.Bacc("TRN2", target_bir_lowering=False, debug=False,
                   num_devices=N_CORES)

    xT = nc.dram_tensor("xT", [C, BT], bf16, kind="ExternalInput")
    wqkvT = nc.dram_tensor("wqkvT", [C, 384], bf16, kind="ExternalInput")
    woT = nc.dram_tensor("woT", [C, 128], bf16, kind="ExternalInput")
    residT = nc.dram_tensor("residT", [128, BT], f16, kind="ExternalInput")
    masks = nc.dram_tensor("masks", [512, 512], bf16, kind="ExternalInput")
    ident = nc.dram_tensor("ident", [128, 128], bf16, kind="ExternalInput")
    outT = nc.dram_tensor("outT", [128, BT], f16, kind="ExternalOutput")

    RG = [list(range(N_CORES))]

    with tile.TileContext(nc) as tc:
        with (
            tc.tile_pool(name="const", bufs=1) as constp,
            tc.tile_pool(name="dram", bufs=1, space="DRAM") as dramp,
            tc.tile_pool(name="qkvout", bufs=1) as qkvp,
            tc.tile_pool(name="ps_st", bufs=2, space="PSUM") as ps_st,
            tc.tile_pool(name="ps_x", bufs=2, space="PSUM") as ps_x,
            tc.tile_pool(name="ps_at", bufs=2, space="PSUM") as ps_at,
        ):
            # ---- constants ----
            masks_sb = constp.tile([128, 4 * 512], bf16)
            for j in range(4):
                nc.sync.dma_start(out=masks_sb[:, j * 512:(j + 1) * 512],
                                  in_=masks[j * 128:(j + 1) * 128, :])
            wqkvT_sb = constp.tile([128, 8 * 384], bf16)
            for ci in range(8):
                nc.sync.dma_start(out=wqkvT_sb[:, ci * 384:(ci + 1) * 384],
                                  in_=wqkvT[ci * 128:(ci + 1) * 128, :])
            woT_sb = constp.tile([128, 8 * 128], bf16)
            for ci in range(8):
                nc.sync.dma_start(out=woT_sb[:, ci * 128:(ci + 1) * 128],
                                  in_=woT[ci * 128:(ci + 1) * 128, :])
            ident_sb = constp.tile([128, 128], bf16)
            nc.sync.dma_start(out=ident_sb[:], in_=ident[:])


            # ---- persistent QKV outputs ----
            QT_sb = qkvp.tile([128, BT], bf16)    # rows 0:64 head even, 64:128 odd
            KT_sb = qkvp.tile([128, BT], bf16)
            V_sb = qkvp.tile([128, 64 * 130], bf16)  # per 128-tok tile: Ve|1|Vo|1
            V_g = V_sb.rearrange("p (t g) -> p t g", g=130)
            nc.vector.memset(V_g[:, :, 64:65], 1.0)
            nc.vector.memset(V_g[:, :, 129:130], 1.0)

            # ---- ag buffers (per batch-half) ----
            ag_in = [[dramp.tile([128, 1024], bf16, name=f"ag_in{b}_{h}")
                      for h in range(2)] for b in range(B)]
            ag_out = [[dramp.tile([1024, 1024], bf16, name=f"ag_out{b}_{h}",
                                  addr_space="Shared") for h in range(2)]
                      for b in range(B)]

            with tc.tile_pool(name="xT", bufs=2) as xtp:
                with (
                    tc.tile_pool(name="pt", bufs=6) as ptp,
                    tc.tile_pool(name="rc", bufs=3) as rcp,
                    tc.tile_pool(name="atn", bufs=4) as atnp,
                    tc.tile_pool(name="rbs", bufs=2) as rbsp,
                    tc.tile_pool(name="ats", bufs=3) as atsp,
                    tc.tile_pool(name="af", bufs=2) as afp,
                    tc.tile_pool(name="vt", bufs=2) as vtp,
                    tc.tile_pool(name="res", bufs=3) as resp,
                    tc.tile_pool(name="os", bufs=4) as osp,
                ):
                    resid_sb = {}
                    xt_sb = {}

                    # ---- emission units -------------------------------
                    # The PE executes its instruction stream in order, so
                    # pure-PE work (QKV projection of the next batch, o-proj
                    # of the previous batch) is chopped into small units and
                    # interleaved into the exp-paced attention stream, where
                    # the PE would otherwise idle waiting on ScalarE.

                    def make_qkv_units(b):
                        tb = b * T
                        units = []

                        def dmas(b=b, tb=tb):
                            xt = xtp.tile([128, 8 * T], bf16, name="xt")
                            xt_sb[b] = xt
                            for w in range(4):
                                for ci in range(8):
                                    nc.sync.dma_start(
                                        out=xt[:, ci * T + w * 512:
                                               ci * T + (w + 1) * 512],
                                        in_=xT[ci * 128:(ci + 1) * 128,
                                               tb + w * 512:tb + (w + 1) * 512])
                            res = resp.tile([128, T], f16, name="res")
                            resid_sb[b] = res
                            nc.sync.dma_start(out=res[:],
                                              in_=residT[:, tb:tb + T])
                        units.append(dmas)

                        state = {}
                        for which, dstname in ((0, "q"), (1, "k")):
                            for tt in range(4):
                                def sub1(b=b, tb=tb, which=which, tt=tt):
                                    ps = ps_x.tile([128, 512], f32, name="x")
                                    state[(which, tt)] = ps
                                    for ci in range(4):
                                        nc.tensor.matmul(
                                            ps[:],
                                            wqkvT_sb[:, ci * 384 + which * 128:
                                                     ci * 384 + which * 128 + 128],
                                            xt_sb[b][:, ci * T + tt * 512:
                                                     ci * T + (tt + 1) * 512],
                                            start=(ci == 0), stop=False)

                                def sub2(b=b, tb=tb, which=which, tt=tt):
                                    ps = state.pop((which, tt))
                                    for ci in range(4, 8):
                                        nc.tensor.matmul(
                                            ps[:],
                                            wqkvT_sb[:, ci * 384 + which * 128:
                                                     ci * 384 + which * 128 + 128],
                                            xt_sb[b][:, ci * T + tt * 512:
                                                     ci * T + (tt + 1) * 512],
                                            start=False, stop=(ci == 7))
                                    dst = QT_sb if which == 0 else KT_sb
                                    nc.vector.tensor_copy(
                                        dst[:, tb + tt * 512:tb + (tt + 1) * 512],
                                        ps[:])
                                units.append(sub1)
                                units.append(sub2)
                        for vt in range(16):
                            def vu(b=b, tb=tb, vt=vt):
                                t64 = b * 16 + vt
                                ps = ps_x.tile([128, 128], f32, name="x")
                                for ci in range(8):
                                    nc.tensor.matmul(
                                        ps[:],
                                        xt_sb[b][:, ci * T + vt * 128:
                                                 ci * T + (vt + 1) * 128],
                                        wqkvT_sb[:, ci * 384 + 256:
                                                 ci * 384 + 384],
                                        start=(ci == 0), stop=(ci == 7))
                                pv = ps.rearrange("p (h e) -> p h e", e=64)
                                dv = V_g[:, t64, :].rearrange(
                                    "p (h e) -> p h e", e=65)[:, :, 0:64]
                                nc.vector.tensor_copy(dv, pv)
                            units.append(vu)
                        return units

                    def make_oproj_parts(b):
                        tb = b * T
                        parts = []
                        opstate = {}
                        for qt in range(4):
                            def af_fn(b=b, qt=qt):
                                af = afp.tile([128, 8 * 512], bf16, name="af")
                                src = ag_out[b][qt // 2].rearrange(
                                    "(c p) q -> p c q", p=128)
                                nc.sync.dma_start(
                                    out=af.rearrange("p (c q) -> p c q", q=512),
                                    in_=src[:, :, (qt % 2) * 512:
                                            (qt % 2) * 512 + 512])
                                opstate[qt] = af

                            def mm_fn(b=b, tb=tb, qt=qt):
                                af = opstate.pop(qt)
                                ps = ps_x.tile([128, 512], f32, name="x")
                                for ci in range(8):
                                    nc.tensor.matmul(
                                        ps[:],
                                        woT_sb[:, ci * 128:(ci + 1) * 128],
                                        af[:, ci * 512:(ci + 1) * 512],
                                        start=(ci == 0), stop=(ci == 7))
                                osb = osp.tile([128, 512], f16, name="os")
                                nc.vector.tensor_add(
                                    osb[:], ps[:],
                                    resid_sb[b][:, qt * 512:(qt + 1) * 512])
                                nc.sync.dma_start(
                                    out=outT[:, tb + qt * 512:
                                             tb + (qt + 1) * 512],
                                    in_=osb[:])
                            parts.append((af_fn, mm_fn))
                        return parts

                    def make_oproj_units(b):
                        units = []
                        for af_fn, mm_fn in make_oproj_parts(b):
                            def u(af_fn=af_fn, mm_fn=mm_fn):
                                af_fn()
                                mm_fn()
                            units.append(u)
                        return units

                    def emit_attention(b, units, force=None):
                        tb = b * T
                        n_slots = 40
                        total = len(units)
                        popped = 0
                        done_kbs = 0

                        def feed(floor=None):
                            nonlocal popped
                            target = (done_kbs * total + n_slots - 1) // n_slots
                            if floor is not None:
                                target = max(target, floor)
                            while popped < min(target, total):
                                fn, min_kb = units[popped]
                                if min_kb > done_kbs:
                                    break
                                fn()
                                popped += 1

                        for qt in range(4):
                            if force and qt in force:
                                feed(floor=force[qt])
                            q0 = tb + qt * 512
                            nkb = 4 * qt + 4
                            ats = atsp.tile([128, 512], bf16, name="ats")

                            def emit_st(kb, qt=qt, q0=q0, tb=tb):
                                k0 = tb + kb * 128
                                st = ps_st.tile([128, 1024], f32, name="st")
                                for half in (0, 1):
                                    p0 = half * 64
                                    nc.tensor.matmul(
                                        st[:, half * 512:half * 512 + 512],
                                        KT_sb[p0:p0 + 64, k0:k0 + 128],
                                        QT_sb[p0:p0 + 64, q0:q0 + 512],
                                        start=True, stop=True)
                                return st

                            at_eo = [ps_at.tile([65, 512], f32, name="at")
                                     for _ in range(2)]
                            sts = [emit_st(0)]
                            if nkb > 1:
                                sts.append(emit_st(1))
                            for kb in range(nkb):
                                t64 = b * 16 + kb
                                diag = kb - 4 * qt
                                st = sts[kb]
                                pt = ptp.tile([128, 1024], bf16, name="pt")
                                nc.scalar.activation(pt[:], st[:], Act.Exp,
                                                     scale=0.125)
                                if diag >= 0:
                                    for half in (0, 1):
                                        nc.vector.tensor_mul(
                                            pt[:, half * 512:half * 512 + 512],
                                            pt[:, half * 512:half * 512 + 512],
                                            masks_sb[:, diag * 512:
                                                     (diag + 1) * 512])
                                if kb + 2 < nkb:
                                    sts.append(emit_st(kb + 2))
                                for half in (0, 1):
                                    nc.tensor.matmul(
                                        at_eo[half][0:65, :],
                                        V_sb[:, t64 * 130 + half * 65:
                                             t64 * 130 + half * 65 + 65],
                                        pt[:, half * 512:half * 512 + 512],
                                        start=(kb == 0), stop=(kb == nkb - 1))
                                done_kbs += 1
                                feed()
                            # normalize + stage for allgather (DVE/GpSimd
                            # only; psum evacuated immediately)
                            for half in (0, 1):
                                p0 = half * 64
                                at = at_eo[half]
                                atn = atnp.tile([65, 512], f32, name="atn")
                                nc.vector.tensor_copy(atn[:], at[0:65, :])
                                den = rcp.tile([1, 512], f32, name="den")
                                nc.vector.tensor_copy(den[:], at[64:65, :])
                                rc = rcp.tile([1, 512], f32, name="rc")
                                nc.vector.reciprocal_approx_fast(rc[:], den[:])
                                rbs = rbsp.tile([64, 512], f32, name="rbs")
                                nc.gpsimd.partition_broadcast(rbs[:], rc[:])
                                nc.vector.tensor_mul(
                                    ats[p0:p0 + 64, :], atn[0:64, :], rbs[:])
                            nc.gpsimd.dma_start(
                                out=ag_in[b][qt // 2][:, (qt % 2) * 512:
                                                      (qt % 2) * 512 + 512],
                                in_=ats[:])
                            if qt % 2 == 1:
                                nc.gpsimd.collective_compute(
                                    "AllGather", Alu.bypass, replica_groups=RG,
                                    ins=[ag_in[b][qt // 2].opt()],
                                    outs=[ag_out[b][qt // 2].opt()])
                        # drain any leftovers
                        while popped < total:
                            units[popped][0]()
                            popped += 1

                    # ---- main schedule --------------------------------
                    # batch 0: emit only the slice of QKV that attention
                    # qt0 needs, feed the rest as units with forced pops at
                    # q-tile boundaries (dependency order).
                    q0units = make_qkv_units(0)
                    for idx in (0, 1, 2, 9, 10, 17, 18, 19, 20):
                        q0units[idx]()
                    rest0 = []
                    for j in (1, 2, 3):
                        rest0 += [q0units[2 * j + 1], q0units[2 * j + 2],
                                  q0units[9 + 2 * j], q0units[10 + 2 * j]]
                        rest0 += q0units[17 + 4 * j:21 + 4 * j]
                    force0 = {1: 8, 2: 16, 3: 24}

                    for b in range(B):
                        units = list(rest0) if b == 0 else []
                        force = force0 if b == 0 else None
                        qkv = make_qkv_units(b + 1) if b + 1 < B else []
                        op = make_oproj_units(b - 1) if b > 0 else []
                        qi = 0
                        for j in range(4):
                            take = min(len(qkv) - qi, 8 + (1 if j == 0 else 0))
                            units += qkv[qi:qi + take]
                            qi += take
                            if j < len(op):
                                units.append(op[j])
                        units += qkv[qi:]
                        units = [(u, 0) for u in units]
                        if b == B - 1:
                            tailp = make_oproj_parts(B - 1)

                            def op3u(qt):
                                def u():
                                    tailp[qt][0]()
                                    tailp[qt][1]()
                                return u
                            units.append((op3u(0), 28))
                            units.append((op3u(1), 33))
                        emit_attention(b, units, force=force)
                    tailp[2][0]()
                    tailp[3][0]()
                    tailp[2][1]()
                    tailp[3][1]()
    nc.compile()
    return nc


def _host_shards(residual, x, w_qkv, w_o):
    import ml_dtypes
    bf16 = ml_dtypes.bfloat16
    xf = np.ascontiguousarray(x.reshape(BT, C).T).astype(bf16)  # (C, BT)
    rf = residual.reshape(BT, C).T                          # (C, BT) view
    woT_full = w_o.T                                        # (C, C) view

    # causal mask tiles: tile j allows key s (0..127) for query q (0..511)
    # when 128*j + s <= q
    jj = np.arange(4)[:, None, None]
    ss = np.arange(128)[None, :, None]
    qq = np.arange(512)[None, None, :]
    masks = ((128 * jj + ss) <= qq).astype(bf16).reshape(512, 512)
    masks = np.ascontiguousarray(masks)
    ident = np.eye(128).astype(bf16)

    in_maps = []
    for c in range(N_CORES):
        r0, r1 = c * 128, (c + 1) * 128
        wq = w_qkv[r0:r1, :]
        wk = w_qkv[C + r0:C + r1, :]
        wv = w_qkv[2 * C + r0:2 * C + r1, :]
        wqkvT = np.ascontiguousarray(
            np.concatenate([wq.T, wk.T, wv.T], axis=1)).astype(bf16)
        in_maps.append({
            "xT": xf,
            "wqkvT": wqkvT,
            "woT": np.ascontiguousarray(woT_full[:, r0:r1]).astype(bf16),
            "residT": np.ascontiguousarray(rf[r0:r1, :]),
            "masks": masks,
            "ident": ident,
        })
    return in_maps


def kernel(residual, x, w_qkv, w_o):
    from concourse.bass_utils import run_bass_kernel_spmd

    residual = np.asarray(residual, dtype=np.float16)
    x = np.asarray(x, dtype=np.float16)
    w_qkv = np.asarray(w_qkv, dtype=np.float16)
    w_o = np.asarray(w_o, dtype=np.float16)

    if "nc" not in _cache:
        _cache["nc"] = _build_graph()
    nc = _cache["nc"]

    in_maps = _host_shards(residual, x, w_qkv, w_o)
    res = run_bass_kernel_spmd(nc, in_maps, core_ids=list(range(N_CORES)),
                               trace=bool(os.environ.get("BASS_TRACE")))
    _cache["last_result"] = res
    outT = np.concatenate([res.results[c]["outT"] for c in range(N_CORES)],
                          axis=0)                           # (C, BT)
    return np.ascontiguousarray(outT.T).reshape(B, T, C)


# revision 35
# speedup vs baseline: 1.0904x; 1.0148x over previous
"""Distributed Bass kernel for causal MHA block (B=4,T=2048,C=1024,H=16,D=64).

Sharding: tensor-parallel over head pairs across 8 cores. Core c owns heads
{2c, 2c+1} and computes QKV+attention for all batches for those heads. The
normalized attention outputs (attnT: head-dims on partitions, tokens free)
are AllGather'd per batch; each core then computes the o-projection for its
128 output channels over all tokens (w_o row-sharded), fused with the
residual add. Host reassembles out = concat(outT_c).T.

Layout notes (everything chosen so the device never transposes):
  - scores are computed transposed (keys on partitions, queries free) so the
    probs tile feeds the PV matmul directly as the moving operand.
  - V carries a fused ones-column; the PV matmul then emits the softmax
    denominator as row 64 of the accumulator for free.
  - softmax skips max-subtraction: scores*0.125 for randn inputs are bounded
    (|s|<~10), exp stays well inside fp16/fp32 range.
"""

import os
import sys

import numpy as np

sys.path.insert(0, "/opt/trn_rl_repo")

B, T, C, H, D = 4, 2048, 1024, 16, 64
BT = B * T  # 8192
N_CORES = 8
F16 = None  # set after mybir import
F32 = None

_cache = {}


def _build_graph():
    import concourse.bacc as bacc
    import concourse.bass as bass
    import concourse.mybir as mybir
    import concourse.tile as tile

    f16 = mybir.dt.float16
    bf16 = mybir.dt.bfloat16
    f32 = mybir.dt.float32
    Alu = mybir.AluOpType
    Act = mybir.ActivationFunctionType

    nc = bacc.Bacc("TRN2", target_bir_lowering=False, debug=False,
                   num_devices=N_CORES)

    xT = nc.dram_tensor("xT", [C, BT], bf16, kind="ExternalInput")
    wqkvT = nc.dram_tensor("wqkvT", [C, 384], bf16, kind="ExternalInput")
    woT = nc.dram_tensor("woT", [C, 128], bf16, kind="ExternalInput")
    residT = nc.dram_tensor("residT", [128, BT], f16, kind="ExternalInput")
    masks = nc.dram_tensor("masks", [512, 512], bf16, kind="ExternalInput")
    ident = nc.dram_tensor("ident", [128, 128], bf16, kind="ExternalInput")
    outT = nc.dram_tensor("outT", [128, BT], f16, kind="ExternalOutput")

    RG = [list(range(N_CORES))]

    with tile.TileContext(nc) as tc:
        with (
            tc.tile_pool(name="const", bufs=1) as constp,
            tc.tile_pool(name="dram", bufs=1, space="DRAM") as dramp,
            tc.tile_pool(name="qkvout", bufs=1) as qkvp,
            tc.tile_pool(name="ps_st", bufs=2, space="PSUM") as ps_st,
            tc.tile_pool(name="ps_x", bufs=2, space="PSUM") as ps_x,
            tc.tile_pool(name="ps_at", bufs=2, space="PSUM") as ps_at,
        ):
            # ---- constants ----
            masks_sb = constp.tile([128, 4 * 512], bf16)
            for j in range(4):
                nc.sync.dma_start(out=masks_sb[:, j * 512:(j + 1) * 512],
                                  in_=masks[j * 128:(j + 1) * 128, :])
            wqkvT_sb = constp.tile([128, 8 * 384], bf16)
            for ci in range(8):
                nc.sync.dma_start(out=wqkvT_sb[:, ci * 384:(ci + 1) * 384],
                                  in_=wqkvT[ci * 128:(ci + 1) * 128, :])
            woT_sb = constp.tile([128, 8 * 128], bf16)
            for ci in range(8):
                nc.sync.dma_start(out=woT_sb[:, ci * 128:(ci + 1) * 128],
                                  in_=woT[ci * 128:(ci + 1) * 128, :])
            ident_sb = constp.tile([128, 128], bf16)
            nc.sync.dma_start(out=ident_sb[:], in_=ident[:])


            # ---- persistent QKV outputs ----
            QT_sb = qkvp.tile([128, BT], bf16)    # rows 0:64 head even, 64:128 odd
            KT_sb = qkvp.tile([128, BT], bf16)
            V_sb = qkvp.tile([128, 64 * 130], bf16)  # per 128-tok tile: Ve|1|Vo|1
            V_g = V_sb.rearrange("p (t g) -> p t g", g=130)
            nc.vector.memset(V_g[:, :, 64:65], 1.0)
            nc.vector.memset(V_g[:, :, 129:130], 1.0)

            # ---- ag buffers (per batch-half) ----
            ag_in = [[dramp.tile([128, 1024], bf16, name=f"ag_in{b}_{h}")
                      for h in range(2)] for b in range(B)]
            ag_out = [[dramp.tile([1024, 1024], bf16, name=f"ag_out{b}_{h}",
                                  addr_space="Shared") for h in range(2)]
                      for b in range(B)]

            with tc.tile_pool(name="xT", bufs=2) as xtp:
                with (
                    tc.tile_pool(name="pt", bufs=6) as ptp,
                    tc.tile_pool(name="rc", bufs=3) as rcp,
                    tc.tile_pool(name="atn", bufs=4) as atnp,
                    tc.tile_pool(name="rbs", bufs=2) as rbsp,
                    tc.tile_pool(name="ats", bufs=3) as atsp,
                    tc.tile_pool(name="af", bufs=2) as afp,
                    tc.tile_pool(name="vt", bufs=2) as vtp,
                    tc.tile_pool(name="res", bufs=3) as resp,
                    tc.tile_pool(name="os", bufs=4) as osp,
                ):
                    resid_sb = {}
                    xt_sb = {}

                    # ---- emission units -------------------------------
                    # The PE executes its instruction stream in order, so
                    # pure-PE work (QKV projection of the next batch, o-proj
                    # of the previous batch) is chopped into small units and
                    # interleaved into the exp-paced attention stream, where
                    # the PE would otherwise idle waiting on ScalarE.

                    def make_qkv_units(b):
                        tb = b * T
                        units = []

                        def dmas(b=b, tb=tb):
                            xt = xtp.tile([128, 8 * T], bf16, name="xt")
                            xt_sb[b] = xt
                            for w in range(4):
                                for ci in range(8):
                                    nc.sync.dma_start(
                                        out=xt[:, ci * T + w * 512:
                                               ci * T + (w + 1) * 512],
                                        in_=xT[ci * 128:(ci + 1) * 128,
                                               tb + w * 512:tb + (w + 1) * 512])
                            res = resp.tile([128, T], f16, name="res")
                            resid_sb[b] = res
                            nc.sync.dma_start(out=res[:],
                                              in_=residT[:, tb:tb + T])
                        units.append(dmas)

                        state = {}
                        for which, dstname in ((0, "q"), (1, "k")):
                            for tt in range(4):
                                def sub1(b=b, tb=tb, which=which, tt=tt):
                                    ps = ps_x.tile([128, 512], f32, name="x")
                                    state[(which, tt)] = ps
                                    for ci in range(4):
                                        nc.tensor.matmul(
                                            ps[:],
                                            wqkvT_sb[:, ci * 384 + which * 128:
                                                     ci * 384 + which * 128 + 128],
                                            xt_sb[b][:, ci * T + tt * 512:
                                                     ci * T + (tt + 1) * 512],
                                            start=(ci == 0), stop=False)

                                def sub2(b=b, tb=tb, which=which, tt=tt):
                                    ps = state.pop((which, tt))
                                    for ci in range(4, 8):
                                        nc.tensor.matmul(
                                            ps[:],
                                            wqkvT_sb[:, ci * 384 + which * 128:
                                                     ci * 384 + which * 128 + 128],
                                            xt_sb[b][:, ci * T + tt * 512:
                                                     ci * T + (tt + 1) * 512],
                                            start=False, stop=(ci == 7))
                                    dst = QT_sb if which == 0 else KT_sb
                                    nc.vector.tensor_copy(
                                        dst[:, tb + tt * 512:tb + (tt + 1) * 512],
                                        ps[:])
                                units.append(sub1)
                                units.append(sub2)
                        for vt in range(16):
                            def vu(b=b, tb=tb, vt=vt):
                                t64 = b * 16 + vt
                                ps = ps_x.tile([128, 128], f32, name="x")
                                for ci in range(8):
                                    nc.tensor.matmul(
                                        ps[:],
                                        xt_sb[b][:, ci * T + vt * 128:
                                                 ci * T + (vt + 1) * 128],
                                        wqkvT_sb[:, ci * 384 + 256:
                                                 ci * 384 + 384],
                                        start=(ci == 0), stop=(ci == 7))
                                pv = ps.rearrange("p (h e) -> p h e", e=64)
                                dv = V_g[:, t64, :].rearrange(
                                    "p (h e) -> p h e", e=65)[:, :, 0:64]
                                nc.vector.tensor_copy(dv, pv)
                            units.append(vu)
                        return units

                    def make_oproj_parts(b):
                        tb = b * T
                        parts = []
                        opstate = {}
                        for qt in range(4):
                            def af_fn(b=b, qt=qt):
                                af = afp.tile([128, 8 * 512], bf16, name="af")
                                src = ag_out[b][qt // 2].rearrange(
                                    "(c p) q -> p c q", p=128)
                                nc.sync.dma_start(
                                    out=af.rearrange("p (c q) -> p c q", q=512),
                                    in_=src[:, :, (qt % 2) * 512:
                                            (qt % 2) * 512 + 512])
                                opstate[qt] = af

                            def mm_fn(b=b, tb=tb, qt=qt):
                                af = opstate.pop(qt)
                                ps = ps_x.tile([128, 512], f32, name="x")
                                for ci in range(8):
                                    nc.tensor.matmul(
                                        ps[:],
                                        woT_sb[:, ci * 128:(ci + 1) * 128],
                                        af[:, ci * 512:(ci + 1) * 512],
                                        start=(ci == 0), stop=(ci == 7))
                                osb = osp.tile([128, 512], f16, name="os")
                                nc.vector.tensor_add(
                                    osb[:], ps[:],
                                    resid_sb[b][:, qt * 512:(qt + 1) * 512])
                                nc.sync.dma_start(
                                    out=outT[:, tb + qt * 512:
                                             tb + (qt + 1) * 512],
                                    in_=osb[:])
                            parts.append((af_fn, mm_fn))
                        return parts

                    def make_oproj_units(b):
                        units = []
                        for af_fn, mm_fn in make_oproj_parts(b):
                            def u(af_fn=af_fn, mm_fn=mm_fn):
                                af_fn()
                                mm_fn()
                            units.append(u)
                        return units

                    def emit_attention(b, units, force=None):
                        tb = b * T
                        n_slots = 40
                        total = len(units)
                        popped = 0
                        done_kbs = 0

                        def feed(floor=None):
                            nonlocal popped
                            target = (done_kbs * total + n_slots - 1) // n_slots
                            if floor is not None:
                                target = max(target, floor)
                            while popped < min(target, total):
                                fn, min_kb = units[popped]
                                if min_kb > done_kbs:
                                    break
                                fn()
                                popped += 1

                        for qt in range(4):
                            if force and qt in force:
                                feed(floor=force[qt])
                            q0 = tb + qt * 512
                            nkb = 4 * qt + 4
                            ats = atsp.tile([128, 512], bf16, name="ats")

                            def emit_st(kb, qt=qt, q0=q0, tb=tb):
                                k0 = tb + kb * 128
                                st = ps_st.tile([128, 1024], f32, name="st")
                                for half in (0, 1):
                                    p0 = half * 64
                                    nc.tensor.matmul(
                                        st[:, half * 512:half * 512 + 512],
                                        KT_sb[p0:p0 + 64, k0:k0 + 128],
                                        QT_sb[p0:p0 + 64, q0:q0 + 512],
                                        start=True, stop=True)
                                return st

                            at_eo = [ps_at.tile([65, 512], f32, name="at")
                                     for _ in range(2)]
                            sts = [emit_st(0)]
                            if nkb > 1:
                                sts.append(emit_st(1))
                            for kb in range(nkb):
                                t64 = b * 16 + kb
                                diag = kb - 4 * qt
                                st = sts[kb]
                                pt = ptp.tile([128, 1024], bf16, name="pt")
                                nc.scalar.activation(pt[:], st[:], Act.Exp,
                                                     scale=0.125)
                                if diag >= 0:
                                    for half in (0, 1):
                                        nc.vector.tensor_mul(
                                            pt[:, half * 512:half * 512 + 512],
                                            pt[:, half * 512:half * 512 + 512],
                                            masks_sb[:, diag * 512:
                                                     (diag + 1) * 512])
                                if kb + 2 < nkb:
                                    sts.append(emit_st(kb + 2))
                                for half in (0, 1):
                                    nc.tensor.matmul(
                                        at_eo[half][0:65, :],
                                        V_sb[:, t64 * 130 + half * 65:
                                             t64 * 130 + half * 65 + 65],
                                        pt[:, half * 512:half * 512 + 512],
                                        start=(kb == 0), stop=(kb == nkb - 1))
                                done_kbs += 1
                                feed()
                            # normalize + stage for allgather (DVE/GpSimd
                            # only; psum evacuated immediately)
                            for half in (0, 1):
                                p0 = half * 64
                                at = at_eo[half]
                                atn = atnp.tile([65, 512], f32, name="atn")
                                nc.vector.tensor_copy(atn[:], at[0:65, :])
                                den = rcp.tile([1, 512], f32, name="den")
                                nc.vector.tensor_copy(den[:], at[64:65, :])
                                rc = rcp.tile([1, 512], f32, name="rc")
                                nc.vector.reciprocal_approx_fast(rc[:], den[:])
                                rbs = rbsp.tile([64, 512], f32, name="rbs")
                                nc.gpsimd.partition_broadcast(rbs[:], rc[:])
                                nc.vector.tensor_mul(
                                    ats[p0:p0 + 64, :], atn[0:64, :], rbs[:])
                            nc.gpsimd.dma_start(
                                out=ag_in[b][qt // 2][:, (qt % 2) * 512:
                                                      (qt % 2) * 512 + 512],
                                in_=ats[:])
                            if qt % 2 == 1:
                                nc.gpsimd.collective_compute(
                                    "AllGather", Alu.bypass, replica_groups=RG,
                                    ins=[ag_in[b][qt // 2].opt()],
                                    outs=[ag_out[b][qt // 2].opt()])
                        # drain any leftovers
                        while popped < total:
                            units[popped][0]()
                            popped += 1

                    # ---- main schedule --------------------------------
                    # batch 0: emit only the slice of QKV that attention
                    # qt0 needs, feed the rest as units with forced pops at
                    # q-tile boundaries (dependency order).
                    q0units = make_qkv_units(0)
                    for idx in (0, 1, 2, 9, 10, 17, 18, 19, 20):
                        q0units[idx]()
                    rest0 = []
                    for j in (1, 2, 3):
                        rest0 += [q0units[2 * j + 1], q0units[2 * j + 2],
                                  q0units[9 + 2 * j], q0units[10 + 2 * j]]
                        rest0 += q0units[17 + 4 * j:21 + 4 * j]
                    force0 = {1: 8, 2: 16, 3: 24}

                    for b in range(B):
                        units = list(rest0) if b == 0 else []
                        force = force0 if b == 0 else None
                        qkv = make_qkv_units(b + 1) if b + 1 < B else []
                        op = make_oproj_units(b - 1) if b > 0 else []
                        qi = 0
                        for j in range(4):
                            take = min(len(qkv) - qi, 8 + (1 if j == 0 else 0))
                            units += qkv[qi:qi + take]
                            qi += take
                            if j < len(op):
                                units.append(op[j])
                        units += qkv[qi:]
                        units = [(u, 0) for u in units]
                        emit_attention(b, units, force=force)
                    tailp = make_oproj_parts(B - 1)
                    tailp[0][0]()
                    for qt in range(4):
                        if qt + 1 < 4:
                            tailp[qt + 1][0]()
                        tailp[qt][1]()
    nc.compile()
    return nc


def _host_shards(residual, x, w_qkv, w_o):
    import ml_dtypes
    bf16 = ml_dtypes.bfloat16
    xf = np.ascontiguousarray(x.reshape(BT, C).T).astype(bf16)  # (C, BT)
    rf = residual.reshape(BT, C).T                          # (C, BT) view
    woT_full = w_o.T                                        # (C, C) view

    # causal mask tiles: tile j allows key s (0..127) for query q (0..511)
    # when 128*j + s <= q
    jj = np.arange(4)[:, None, None]
    ss = np.arange(128)[None, :, None]
    qq = np.arange(512)[None, None, :]
    masks = ((128 * jj + ss) <= qq).astype(bf16).reshape(512, 512)
    masks = np.ascontiguousarray(masks)
    ident = np.eye(128).astype(bf16)

    in_maps = []
    for c in range(N_CORES):
        r0, r1 = c * 128, (c + 1) * 128
        wq = w_qkv[r0:r1, :]
        wk = w_qkv[C + r0:C + r1, :]
        wv = w_qkv[2 * C + r0:2 * C + r1, :]
        wqkvT = np.ascontiguousarray(
            np.concatenate([wq.T, wk.T, wv.T], axis=1)).astype(bf16)
        in_maps.append({
            "xT": xf,
            "wqkvT": wqkvT,
            "woT": np.ascontiguousarray(woT_full[:, r0:r1]).astype(bf16),
            "residT": np.ascontiguousarray(rf[r0:r1, :]),
            "masks": masks,
            "ident": ident,
        })
    return in_maps


def kernel(residual, x, w_qkv, w_o):
    from concourse.bass_utils import run_bass_kernel_spmd

    residual = np.asarray(residual, dtype=np.float16)
    x = np.asarray(x, dtype=np.float16)
    w_qkv = np.asarray(w_qkv, dtype=np.float16)
    w_o = np.asarray(w_o, dtype=np.float16)

    if "nc" not in _cache:
        _cache["nc"] = _build_graph()
    nc = _cache["nc"]

    in_maps = _host_shards(residual, x, w_qkv, w_o)
    res = run_bass_kernel_spmd(nc, in_maps, core_ids=list(range(N_CORES)),
                               trace=bool(os.environ.get("BASS_TRACE")))
    _cache["last_result"] = res
    outT = np.concatenate([res.results[c]["outT"] for c in range(N_CORES)],
                          axis=0)                           # (C, BT)
    return np.ascontiguousarray(outT.T).reshape(B, T, C)
